# revision 13
# baseline (speedup 1.0000x reference)
"""Trainium2 Bass kernel for the GraphCast-style grid->mesh Encoder.

Strategy (8 NeuronCores, no collectives):
  - Mesh nodes (the only segment_sum targets) are sharded 736/core; each core
    owns every edge targeting its mesh shard.  Edges are grouped by aligned
    128-mesh-node blocks (6 per core) and padded to a uniform per-block tile
    count so all cores run one SPMD program.
  - Grid nodes are sharded 32400/core for the node-encoder + node-update path
    (their segment aggregation is exactly zero: dst only hits mesh nodes).
  - Edge sources are covered by recomputing the node encoder on host-gathered
    features[src] per edge slot - cheaper than any halo exchange.
  - Gather of x[dst] (through the W1b fold: G = x_mesh @ W1b) and the
    scatter-add of edge outputs both become small one-hot matmuls against
    host-built bf16 one-hot tiles over the 128-wide mesh window of each block.

Layouts: matmul chains run "A-form" (features on partitions, rows on the free
axis, inputs host-pre-transposed); each MLP's final layer runs "B-form"
(rows on partitions) so LayerNorm reduces along the free axis; PE transposes
bridge the two junctions.  All big matmuls use float32r (full PE rate at
moving-dim >= 256).
"""

import numpy as np
import ml_dtypes

import concourse.bass as bass
import concourse.mybir as mybir
import concourse.tile as tile
from concourse.vector_clock import ScopedClock
from concourse.masks import make_identity
from concourse.bass_utils import run_bass_kernel_spmd

F32 = mybir.dt.float32
F32R = mybir.dt.float32r
BF16 = mybir.dt.bfloat16
RELU = mybir.ActivationFunctionType.Relu

N_GRID = 360 * 720            # 259200
N_H3 = 5882
N_NODES = N_GRID + N_H3       # 265082
E_TOT = N_GRID
IN_DIM = 78
HID = 256
NCORES = 8
GN = N_GRID // NCORES         # 32400 grid rows per core
MESH_PER = 736                # mesh rows per core (8*736 = 5888 >= 5882)
MESH_PAD = 768                # 6 blocks of 128
NBLK = MESH_PAD // 128        # 6
T = 512                       # row tile (free dim)
EPS = 1e-5

_PATCHED = False


def _patch_tile_drain():
    """This walrus build only accepts one semaphore wait per CTRL
    instruction; TileContext's kernel-tail drain aggregates every
    end-of-kernel wait onto a single drain.  Split it: one drain per wait."""
    global _PATCHED
    if _PATCHED:
        return
    _PATCHED = True

    def _drain_and_barrier(self, tick_clock, wait_clock):
        nc = self.nc
        drain_inst = nc.sync.drain()
        wait_clock.add_sem_waits(
            drain_inst.ins, ScopedClock({None: tick_clock.global_clock})
        )
        si = drain_inst.ins.sync_info
        waits = list(si.on_wait) if si is not None and si.on_wait else []
        if len(waits) > 1:
            del si.on_wait[1:]
            for w in waits[1:]:
                d2 = nc.sync.drain()
                d2.ins.sync_info = mybir.SyncInfo(on_wait=[w], on_update=[])
        nc.all_engine_barrier()
        popped = nc._tile_sem_poison_stack.pop()
        assert popped is self._sem_poison
        nc.clear_and_free_semaphores(list(self.sems.allocated().values()))
        nc.all_engine_barrier()

    tile.TileContext._drain_and_barrier = _drain_and_barrier


def r32(ap):
    return ap.bitcast(F32R)


def _split_multi_waits(nc):
    """This walrus build accepts at most one semaphore wait per instruction.
    Hoist extra waits onto same-engine nops inserted just before."""
    for bb in nc.main_func.blocks:
        insts = bb.instructions
        out = []
        changed = False
        for inst in insts:
            si = inst.sync_info
            if si is not None and si.on_wait and len(si.on_wait) > 1:
                waits = list(si.on_wait)
                del si.on_wait[1:]
                for w in waits[1:]:
                    nop = nc.engines[inst.engine].nop(
                        hint="waitsplit", nofuse=True).ins
                    # pop the freshly appended nop from wherever it landed
                    for bb2 in nc.main_func.blocks:
                        if bb2.instructions and bb2.instructions[-1] is nop:
                            bb2.instructions.pop()
                            break
                    nop.sync_info = mybir.SyncInfo(on_wait=[w], on_update=[])
                    out.append(nop)
                changed = True
            out.append(inst)
        if changed:
            insts[:] = out


# ----------------------------------------------------------------------------
# device program
# ----------------------------------------------------------------------------

def build_program(grid_tiles, b_tiles, ln2_consts, emit_grid=None,
                  emit_bt=None):
    """Emit the per-core SPMD program.

    grid_tiles: number of 512-row grid tiles per core.
    b_tiles:    edge tiles (512 edges) per mesh block; 6*b_tiles tiles total.
    ln2_consts: dict with float (gamma0, gamma1, beta0, beta1, bdiff) for the
                two dim-2 LayerNorms keyed 'e' (edge encoder) and 'ep'
                (edge processor), plus 'e_b' / 'ep_b' layer-3 bias pairs.
    """
    _patch_tile_drain()
    nt_edges = NBLK * b_tiles
    etot = nt_edges * T
    g_emit = grid_tiles if emit_grid is None else emit_grid
    bt_emit = b_tiles if emit_bt is None else emit_bt

    nc = bass.Bass(target_bir_lowering=False)

    # ---- DRAM I/O ------------------------------------------------------
    d = {}

    def din(name, shape, dtype=F32):
        d[name] = nc.dram_tensor(name, list(shape), dtype, kind="ExternalInput")
        return d[name]

    din("featT_grid", [IN_DIM, grid_tiles * T], F32R)
    din("featT_mesh", [IN_DIM, MESH_PAD], F32R)
    din("featT_src", [IN_DIM, etot], F32R)
    din("eattrT", [2, etot], F32R)
    din("ohT", [nt_edges, 128, T], F32R)        # [mesh-window, edge] per tile
    din("ohE", [nt_edges, 128, 4, 128], BF16)   # [edge-in-sub, sub, window]

    # weights (kc-chunked [2,128,N] where contracting dim is 256)
    for nm in ("w1n", "b1n_c", "w2n", "b2n_c", "w3n",
               "v1a", "v1b", "b1np_c", "v2np", "b2np_c", "v3np",
               "we1", "be1_c", "we2", "be2_c", "we3",
               "w1a", "w1b", "w1c_ext", "w2ep", "b2ep_c", "w3ep"):
        pass  # shapes given at call sites below
    din("w1n", [IN_DIM, HID], F32R)
    din("b1n_c", [128, 2])
    din("w2n", [2, 128, HID], F32R)
    din("b2n_c", [128, 2])
    din("w3n", [2, 128, HID], F32R)
    din("ln_n_b3", [128, HID])     # bias3 broadcast
    din("ln_n_g", [128, HID])
    din("ln_n_be", [128, HID])
    din("v1a", [2, 128, HID], F32R)
    din("v1b", [2, HID], F32R)
    din("b1np_c", [128, 2])
    din("v2np", [2, 128, HID], F32R)
    din("b2np_c", [128, 2])
    din("v3np", [2, 128, HID], F32R)
    din("ln_np_b3", [128, HID])
    din("ln_np_g", [128, HID])
    din("ln_np_be", [128, HID])
    din("we1", [2, HID], F32R)
    din("be1_c", [128, 2])
    din("we2", [2, 128, HID], F32R)
    din("be2_c", [128, 2])
    din("we3", [2, 128, 2], F32R)
    din("w1a", [2, 128, HID], F32R)
    din("w1b", [2, 128, HID], F32R)
    din("w1c", [2, HID], F32R)
    din("b1ep_c", [128, 2])
    din("w2ep", [2, 128, HID], F32R)
    din("b2ep_c", [128, 2])
    din("w3ep", [2, 128, 2], F32R)

    out_grid = nc.dram_tensor("out_grid", [grid_tiles * T, HID], F32,
                              kind="ExternalOutput")
    out_mesh = nc.dram_tensor("out_mesh", [MESH_PAD, HID], F32,
                              kind="ExternalOutput")

    from contextlib import ExitStack
    with tile.TileContext(nc) as tc, ExitStack() as ctx:
        consts = ctx.enter_context(tc.tile_pool(name="consts", bufs=1))
        ps_a = ctx.enter_context(tc.tile_pool(name="ps_a", bufs=2, space="PSUM"))
        ps_b = ctx.enter_context(tc.tile_pool(name="ps_b", bufs=2, space="PSUM"))
        ps_t = ctx.enter_context(tc.tile_pool(name="ps_t", bufs=3, space="PSUM"))
        psagg = ctx.enter_context(tc.tile_pool(name="psagg", bufs=1, space="PSUM"))
        pbig = ctx.enter_context(tc.tile_pool(name="pbig", bufs=6))
        pxt = ctx.enter_context(tc.tile_pool(name="pxt", bufs=2))
        pxb = ctx.enter_context(tc.tile_pool(name="pxb", bufs=2))
        pool = ctx.enter_context(tc.tile_pool(name="work", bufs=3))
        opool = ctx.enter_context(tc.tile_pool(name="outp", bufs=3))

        # ---- load constants into SBUF ---------------------------------
        sb = {}

        def load(name, shape, dtype=F32):
            t_ = consts.tile(list(shape), dtype, tag=name)
            nc.sync.dma_start(out=t_[:], in_=d[name][:])
            sb[name] = t_
            return t_

        load("w1n", [IN_DIM, HID], F32R)
        load("b1n_c", [128, 2])

        def load_k2(name, n=HID):
            t_ = consts.tile([128, 2, n], F32R, tag=name)
            for kc in range(2):
                nc.sync.dma_start(out=t_[:, kc, :], in_=d[name][kc])
            sb[name] = t_
            return t_

        load_k2("w2n")
        load_k2("w3n")
        load_k2("v1a")
        load_k2("v2np")
        load_k2("v3np")
        load_k2("we2")
        load_k2("w1a")
        load_k2("w1b")
        load_k2("w2ep")
        load_k2("we3", n=2)
        load_k2("w3ep", n=2)
        for row in (
            ("b2n_c", [128, 2]), ("v1b", [2, HID], F32R), ("b1np_c", [128, 2]),
            ("b2np_c", [128, 2]), ("we1", [2, HID], F32R), ("be1_c", [128, 2]),
            ("be2_c", [128, 2]), ("w1c", [2, HID], F32R), ("b1ep_c", [128, 2]), ("b2ep_c", [128, 2]),
            ("ln_n_b3", [128, HID]), ("ln_n_g", [128, HID]),
            ("ln_n_be", [128, HID]), ("ln_np_b3", [128, HID]),
            ("ln_np_g", [128, HID]), ("ln_np_be", [128, HID]),
        ):
            load(*row)

        ident = consts.tile([128, 128], F32, tag="ident")
        make_identity(nc, ident[:])

        # persistent mesh state
        x_meshT = consts.tile([128, 2, MESH_PAD], F32R, tag="x_meshT")
        x_meshB = consts.tile([128, NBLK, HID], F32, tag="x_meshB")
        g_tab = consts.tile([128, NBLK, HID], F32R, tag="g_tab")
        aggT = consts.tile([2, MESH_PAD], F32R, tag="aggT")

        engines = [nc.vector, nc.scalar]

        # ---- helpers ---------------------------------------------------
        def evict_relu(ps_ap, out_ap, bias_ap, eng_i):
            """out = relu(ps + bias); bias is [128,1] per-partition."""
            if eng_i % 2 == 0:
                nc.vector.tensor_scalar(
                    out=out_ap, in0=ps_ap, scalar1=bias_ap, scalar2=0.0,
                    op0=mybir.AluOpType.add, op1=mybir.AluOpType.max)
            else:
                nc.scalar.activation(out_ap, ps_ap, RELU, bias=bias_ap)

        def layerA(rhs, kchunks, w_name, bias_name, out_t, TT, first_k=IN_DIM):
            """A-form layer: out[128,2,TT] = relu(W.T @ rhs + b).
            rhs: [K-part, kchunks, TT] sbuf (kchunks==1 -> [K, TT]).
            """
            w = sb[w_name]
            for fc in range(2):
                ps = ps_a.tile([128, T], F32, tag="psA")
                for kc in range(kchunks):
                    lhsT = (w[:, fc * 128:(fc + 1) * 128] if kchunks == 1
                            else w[:, kc, fc * 128:(fc + 1) * 128])
                    rr = rhs[:, :TT] if kchunks == 1 else rhs[:, kc, :TT]
                    nc.tensor.matmul(ps[:, :TT], r32(lhsT), r32(rr),
                                     start=(kc == 0), stop=(kc == kchunks - 1))
                evict_relu(ps[:, :TT], out_t[:, fc, :TT],
                           sb[bias_name][:, fc:fc + 1], fc)
            return out_t

        def ln_full(ps_ap, b3, g, be, out_ap, resid_ap=None, TTp=128):
            """LayerNorm(ps + b3-row) * gamma + beta (+resid) in B-form.
            ps_ap: [TTp,256] psum.  b3/g/be: [128,256] broadcast consts."""
            z = pool.tile([128, HID], F32, tag="ln_z")
            nc.vector.tensor_add(z[:TTp], ps_ap, sb[b3][:TTp])
            stats = pool.tile([128, 6], F32, tag="ln_stats")
            nc.vector.bn_stats(out=stats[:TTp], in_=z[:TTp])
            mv = pool.tile([128, 2], F32, tag="ln_mv")
            nc.vector.bn_aggr(out=mv[:TTp], in_=stats[:TTp])
            veps = pool.tile([128, 1], F32, tag="ln_veps")
            nc.vector.tensor_scalar_add(veps[:TTp], mv[:TTp, 1:2], EPS)
            sq = pool.tile([128, 1], F32, tag="ln_sq")
            nc.scalar.sqrt(sq[:TTp], veps[:TTp])
            rsig = pool.tile([128, 1], F32, tag="ln_rsig")
            nc.vector.reciprocal(rsig[:TTp], sq[:TTp])
            zn = pool.tile([128, HID], F32, tag="ln_zn")
            nc.vector.tensor_scalar(
                out=zn[:TTp], in0=z[:TTp], scalar1=mv[:TTp, 0:1],
                scalar2=rsig[:TTp], op0=mybir.AluOpType.subtract,
                op1=mybir.AluOpType.mult)
            nc.vector.tensor_mul(zn[:TTp], zn[:TTp], sb[g][:TTp])
            if resid_ap is not None:
                nc.vector.tensor_add(zn[:TTp], zn[:TTp], sb[be][:TTp])
                nc.vector.tensor_add(out_ap, zn[:TTp], resid_ap)
            else:
                nc.vector.tensor_add(out_ap, zn[:TTp], sb[be][:TTp])

        def ln2(psE, c, out_t, resid_ap=None, out_bf=None):
            """Analytic dim-2 LayerNorm on [128,4,2] (+bias3 pair folded).
            c = (g0, g1, be0, be1, bd) with bd = (b3[0]-b3[1])/2."""
            g0, g1, be0, be1, bd = c
            z = pool.tile([128, 4, 2], F32, tag="l2_z")
            nc.vector.tensor_copy(z[:], psE)
            dd = pool.tile([128, 4], F32, tag="l2_d")
            # d = (z0 - z1)*0.5 + bd
            nc.vector.tensor_sub(dd[:], z[:, :, 0], z[:, :, 1])
            nc.vector.tensor_scalar(
                out=dd[:], in0=dd[:], scalar1=0.5, scalar2=bd,
                op0=mybir.AluOpType.mult, op1=mybir.AluOpType.add)
            tt = pool.tile([128, 4], F32, tag="l2_t")
            nc.vector.tensor_mul(tt[:], dd[:], dd[:])
            nc.vector.tensor_scalar_add(tt[:], tt[:], EPS)
            nc.scalar.sqrt(tt[:], tt[:])
            rs = pool.tile([128, 4], F32, tag="l2_rs")
            nc.vector.reciprocal(rs[:], tt[:])
            nn_ = pool.tile([128, 4], F32, tag="l2_n")
            nc.vector.tensor_mul(nn_[:], dd[:], rs[:])
            nc.vector.tensor_scalar(
                out=out_t[:, :, 0], in0=nn_[:], scalar1=g0, scalar2=be0,
                op0=mybir.AluOpType.mult, op1=mybir.AluOpType.add)
            nc.vector.tensor_scalar(
                out=out_t[:, :, 1], in0=nn_[:], scalar1=-g1, scalar2=be1,
                op0=mybir.AluOpType.mult, op1=mybir.AluOpType.add)
            if resid_ap is not None:
                nc.vector.tensor_add(out_t[:], out_t[:], resid_ap)
            if out_bf is not None:
                nc.vector.tensor_copy(out_bf[:], out_t[:])

        def node_mlp_to_B(featT_ap, TT, xB_t, xT_t=None):
            """node encoder (78->256->256->256 + LN) for TT rows.
            featT_ap: [78, TT].  xB_t: [128, TT//128, 256] out (post-LN).
            xT_t (optional): [128, 2, TT] transposed copy of the output."""
            nsub = TT // 128
            h1 = pbig.tile([128, 2, T], F32R, tag="hA")
            layerA(featT_ap, 1, "w1n", "b1n_c", h1, TT)
            h2 = pbig.tile([128, 2, T], F32R, tag="hA")
            layerA(h1, 2, "w2n", "b2n_c", h2, TT)
            for m in range(nsub):
                ps = ps_b.tile([128, HID], F32, tag="psB")
                for kc in range(2):
                    nc.tensor.matmul(
                        ps[:], r32(h2[:, kc, m * 128:(m + 1) * 128]),
                        r32(sb["w3n"][:, kc, :]),
                        start=(kc == 0), stop=(kc == 1))
                ln_full(ps[:], "ln_n_b3", "ln_n_g", "ln_n_be",
                        xB_t[:, m, :])
            if xT_t is not None:
                for c in range(2):
                    pst = ps_t.tile([128, T], F32, tag="pst")
                    for m in range(nsub):
                        nc.tensor.transpose(
                            pst[:, m * 128:(m + 1) * 128],
                            xB_t[:, m, c * 128:(c + 1) * 128], ident[:])
                    nc.vector.tensor_copy(xT_t[:, c, :TT], pst[:, :TT])
            return xB_t

        # =================================================================
        # mesh prologue: x_mesh (B+T forms) and G table
        # =================================================================
        for tt, TT in ((0, T), (1, MESH_PAD - T)):
            featTm = pool.tile([IN_DIM, T], F32R, tag="featm")
            nc.sync.dma_start(out=featTm[:, :TT],
                              in_=d["featT_mesh"][:, tt * T: tt * T + TT])
            nsub = TT // 128
            xB_view = x_meshB[:, tt * 4: tt * 4 + nsub, :]
            xT_view = x_meshT[:, :, tt * T: tt * T + TT]
            node_mlp_to_B(featTm[:, :TT], TT, xB_view, xT_view)

        for b in range(NBLK):
            ps = ps_b.tile([128, HID], F32, tag="psB")
            for kc in range(2):
                nc.tensor.matmul(
                    ps[:], r32(x_meshT[:, kc, b * 128:(b + 1) * 128]),
                    r32(sb["w1b"][:, kc, :]), start=(kc == 0), stop=(kc == 1))
            nc.vector.tensor_copy(g_tab[:, b, :], ps[:])

        # =================================================================
        # edge pipeline
        # =================================================================
        for b in range(NBLK):
            psA = psagg.tile([2, 128], F32, tag="agg")
            for ti in range(bt_emit):
                t_ = b * b_tiles + ti
                # --- sources: node encoder -> B form -> transpose -------
                featTs = pool.tile([IN_DIM, T], F32R, tag="feats")
                nc.sync.dma_start(out=featTs[:],
                                  in_=d["featT_src"][:, t_ * T:(t_ + 1) * T])
                xsB = pxb.tile([128, 4, HID], F32, tag="xB")
                xsT = pxt.tile([128, 2, T], F32R, tag="xT")
                node_mlp_to_B(featTs[:], T, xsB, xsT)

                # --- edge encoder --------------------------------------
                eattr = pool.tile([2, T], F32R, tag="eattr")
                nc.sync.dma_start(out=eattr[:],
                                  in_=d["eattrT"][:, t_ * T:(t_ + 1) * T])
                e1 = pbig.tile([128, 2, T], F32R, tag="hA")
                layerA(eattr, 1, "we1", "be1_c", e1, T)
                e2 = pbig.tile([128, 2, T], F32R, tag="hA")
                layerA(e1, 2, "we2", "be2_c", e2, T)
                psE = ps_t.tile([128, T], F32, tag="pst")
                for s in range(4):
                    for kc in range(2):
                        nc.tensor.matmul(
                            psE[:, s * 2:(s + 1) * 2],
                            r32(e2[:, kc, s * 128:(s + 1) * 128]),
                            r32(sb["we3"][:, kc, :]),
                            start=(kc == 0), stop=(kc == 1))
                eB = pool.tile([128, 4, 2], F32, tag="eB")
                ln2(psE[:, 0:8].rearrange("p (s d) -> p s d", d=2),
                    ln2_consts["e"], eB)

                # eT_ext = [e0; e1; ones] in A-form via PE transpose
                psT2 = ps_t.tile([128, T], F32, tag="pst")
                for s in range(4):
                    nc.tensor.transpose(
                        psT2[0:2, s * 128:(s + 1) * 128],
                        eB[:, s, :], ident[:])
                eTx = pool.tile([2, T], F32R, tag="eTx")
                nc.vector.tensor_copy(eTx[:], psT2[0:2, :])

                # --- one-hot tiles -------------------------------------
                ohT = pool.tile([128, T], F32R, tag="ohT")
                nc.sync.dma_start(out=ohT[:], in_=d["ohT"][t_])
                ohE = pool.tile([128, 4, 128], BF16, tag="ohE")
                nc.sync.dma_start(out=ohE[:], in_=d["ohE"][t_])

                # --- h1 = relu(xs@W1a + G[dst] + e@W1c + b1) -----------
                h1 = pbig.tile([128, 2, T], F32R, tag="hA")
                for fc in range(2):
                    ps = ps_a.tile([128, T], F32, tag="psA")
                    for kc in range(2):
                        nc.tensor.matmul(
                            ps[:], r32(sb["w1a"][:, kc, fc * 128:(fc + 1) * 128]),
                            r32(xsT[:, kc, :]), start=(kc == 0), stop=False)
                    nc.tensor.matmul(
                        ps[:], g_tab[:, b, fc * 128:(fc + 1) * 128],
                        ohT[:], start=False, stop=False)
                    nc.tensor.matmul(
                        ps[:], r32(sb["w1c"][:, fc * 128:(fc + 1) * 128]),
                        r32(eTx[:]), start=False, stop=True)
                    evict_relu(ps[:], h1[:, fc, :],
                               sb["b1ep_c"][:, fc:fc + 1], fc)

                h2 = pbig.tile([128, 2, T], F32R, tag="hA")
                layerA(h1, 2, "w2ep", "b2ep_c", h2, T)

                psE2 = ps_t.tile([128, T], F32, tag="pst")
                for s in range(4):
                    for kc in range(2):
                        nc.tensor.matmul(
                            psE2[:, s * 2:(s + 1) * 2],
                            r32(h2[:, kc, s * 128:(s + 1) * 128]),
                            r32(sb["w3ep"][:, kc, :]),
                            start=(kc == 0), stop=(kc == 1))
                enew = pool.tile([128, 4, 2], F32, tag="enew")
                enb = pool.tile([128, 4, 2], BF16, tag="enb")
                ln2(psE2[:, 0:8].rearrange("p (s d) -> p s d", d=2),
                    ln2_consts["ep"], enew, resid_ap=eB[:], out_bf=enb)

                # --- scatter-add into the block aggregate --------------
                for s in range(4):
                    nc.tensor.matmul(
                        psA[:], enb[:, s, :], ohE[:, s, :],
                        start=(ti == 0 and s == 0),
                        stop=(ti == bt_emit - 1 and s == 3))
            nc.vector.tensor_copy(aggT[:, b * 128:(b + 1) * 128], psA[:])

        # =================================================================
        # grid pipeline
        # =================================================================
        for i in range(g_emit):
            featT = pool.tile([IN_DIM, T], F32R, tag="featg")
            nc.sync.dma_start(out=featT[:],
                              in_=d["featT_grid"][:, i * T:(i + 1) * T])
            xB = pxb.tile([128, 4, HID], F32, tag="xB")
            xT = pxt.tile([128, 2, T], F32R, tag="xT")
            node_mlp_to_B(featT[:], T, xB, xT)

            p1 = pbig.tile([128, 2, T], F32R, tag="hA")
            layerA(xT, 2, "v1a", "b1np_c", p1, T)
            p2 = pbig.tile([128, 2, T], F32R, tag="hA")
            layerA(p1, 2, "v2np", "b2np_c", p2, T)
            for m in range(4):
                ps = ps_b.tile([128, HID], F32, tag="psB")
                for kc in range(2):
                    nc.tensor.matmul(
                        ps[:], r32(p2[:, kc, m * 128:(m + 1) * 128]),
                        r32(sb["v3np"][:, kc, :]), start=(kc == 0),
                        stop=(kc == 1))
                ot = opool.tile([128, HID], F32, tag="g_out")
                ln_full(ps[:], "ln_np_b3", "ln_np_g", "ln_np_be",
                        ot[:], resid_ap=xB[:, m, :])
                nc.sync.dma_start(
                    out=out_grid[i * T + m * 128: i * T + (m + 1) * 128, :],
                    in_=ot[:])

        # =================================================================
        # mesh epilogue: node update for own mesh rows
        # =================================================================
        for tt, TT in ((0, T), (1, MESH_PAD - T)):
            off = tt * T
            m1 = pbig.tile([128, 2, T], F32R, tag="hA")
            for fc in range(2):
                ps = ps_a.tile([128, T], F32, tag="psA")
                for kc in range(2):
                    nc.tensor.matmul(
                        ps[:, :TT],
                        r32(sb["v1a"][:, kc, fc * 128:(fc + 1) * 128]),
                        r32(x_meshT[:, kc, off:off + TT]),
                        start=(kc == 0), stop=False)
                nc.tensor.matmul(
                    ps[:, :TT], r32(sb["v1b"][:, fc * 128:(fc + 1) * 128]),
                    r32(aggT[:, off:off + TT]), start=False, stop=True)
                evict_relu(ps[:, :TT], m1[:, fc, :TT],
                           sb["b1np_c"][:, fc:fc + 1], fc)
            m2 = pbig.tile([128, 2, T], F32R, tag="hA")
            layerA(m1, 2, "v2np", "b2np_c", m2, TT)
            for m in range(TT // 128):
                ps = ps_b.tile([128, HID], F32, tag="psB")
                for kc in range(2):
                    nc.tensor.matmul(
                        ps[:], r32(m2[:, kc, m * 128:(m + 1) * 128]),
                        r32(sb["v3np"][:, kc, :]), start=(kc == 0),
                        stop=(kc == 1))
                ot = opool.tile([128, HID], F32, tag="m_out")
                ln_full(ps[:], "ln_np_b3", "ln_np_g", "ln_np_be",
                        ot[:], resid_ap=x_meshB[:, tt * 4 + m, :])
                nc.sync.dma_start(
                    out=out_mesh[off + m * 128: off + (m + 1) * 128, :],
                    in_=ot[:])

    _split_multi_waits(nc)
    return nc


# ----------------------------------------------------------------------------
# host side
# ----------------------------------------------------------------------------

def _np(a):
    return np.asarray(a, dtype=np.float32) if np.asarray(a).dtype != np.float32 \
        else np.asarray(a)


def host_prep(features, edge_attr, node_enc, edge_enc, ep_mlp, np_mlp,
              src, dst, grid_tiles, n_grid=N_GRID):
    """Build per-core input maps.  Returns (in_maps, b_tiles, meta)."""
    features = np.asarray(features, np.float32)
    edge_attr = np.asarray(edge_attr, np.float32)
    src = np.asarray(src, np.int64)
    dst = np.asarray(dst, np.int64)
    n_mesh = features.shape[0] - n_grid
    gpc = grid_tiles * T          # padded grid rows per core
    gn = n_grid // NCORES

    assert dst.min() >= n_grid and dst.max() < features.shape[0], \
        "kernel assumes dst targets mesh nodes only"

    dloc = dst - n_grid
    owner = np.minimum(dloc // MESH_PER, NCORES - 1)
    blk = (dloc - owner * MESH_PER) // 128
    w = (dloc - owner * MESH_PER) % 128

    # per (core, block) counts -> uniform tile count
    counts = np.zeros((NCORES, NBLK), np.int64)
    np.add.at(counts, (owner, blk), 1)
    b_tiles = max(1, int(-(-counts.max() // T)))
    b_slots = b_tiles * T
    etot = NBLK * b_slots

    # weights
    def mlp(params):
        p = [np.asarray(x, np.float32) for x in params]
        return p  # [W1,b1,W2,b2,W3,b3,gamma,beta]

    ne, ee, ep, npm = mlp(node_enc), mlp(edge_enc), mlp(ep_mlp), mlp(np_mlp)

    def col2(b):  # [256] -> [128,2]
        return np.ascontiguousarray(b.reshape(2, 128).T)

    def k2(w):    # [256,N] -> [2,128,N]
        return np.ascontiguousarray(w.reshape(2, 128, -1))

    def bc(v):    # [256] -> [128,256]
        return np.ascontiguousarray(np.tile(v[None, :], (128, 1)))

    shared = {
        "w1n": ne[0], "b1n_c": col2(ne[1]), "w2n": k2(ne[2]),
        "b2n_c": col2(ne[3]), "w3n": k2(ne[4]),
        "ln_n_b3": bc(ne[5]), "ln_n_g": bc(ne[6]), "ln_n_be": bc(ne[7]),
        "v1a": k2(npm[0][:256]), "v1b": np.ascontiguousarray(npm[0][256:258]),
        "b1np_c": col2(npm[1]), "v2np": k2(npm[2]), "b2np_c": col2(npm[3]),
        "v3np": k2(npm[4]),
        "ln_np_b3": bc(npm[5]), "ln_np_g": bc(npm[6]), "ln_np_be": bc(npm[7]),
        "we1": ee[0], "be1_c": col2(ee[1]), "we2": k2(ee[2]),
        "be2_c": col2(ee[3]), "we3": k2(ee[4]),
        "w1a": k2(ep[0][:256]), "w1b": k2(ep[0][256:512]),
        "w1c": np.ascontiguousarray(ep[0][512:514]),
        "b1ep_c": col2(ep[1]),
        "w2ep": k2(ep[2]), "b2ep_c": col2(ep[3]), "w3ep": k2(ep[4]),
    }
    shared = {k: np.ascontiguousarray(v, np.float32) for k, v in shared.items()}

    ln2c = {
        "e": (float(ee[6][0]), float(ee[6][1]), float(ee[7][0]),
              float(ee[7][1]), float(ee[5][0] - ee[5][1]) * 0.5),
        "ep": (float(ep[6][0]), float(ep[6][1]), float(ep[7][0]),
               float(ep[7][1]), float(ep[5][0] - ep[5][1]) * 0.5),
    }

    feat_mesh = np.zeros((MESH_PAD, IN_DIM), np.float32)

    in_maps = []
    for k in range(NCORES):
        # grid features
        fg = np.zeros((gpc, IN_DIM), np.float32)
        fg[:gn] = features[k * gn:(k + 1) * gn]
        # mesh features for own shard
        fm = np.zeros((MESH_PAD, IN_DIM), np.float32)
        lo = n_grid + k * MESH_PER
        hi = min(n_grid + (k + 1) * MESH_PER, features.shape[0])
        if hi > lo:
            fm[:hi - lo] = features[lo:hi]
        # edges of this core
        sel = np.nonzero(owner == k)[0]
        bsel = blk[sel]
        slot_rank = np.zeros(len(sel), np.int64)
        for b in range(NBLK):
            m = bsel == b
            slot_rank[m] = np.arange(m.sum())
        slots = bsel * b_slots + slot_rank
        fs = np.zeros((etot, IN_DIM), np.float32)
        fs[slots] = features[src[sel]]
        ea = np.zeros((etot, 2), np.float32)
        ea[slots] = edge_attr[sel]
        tidx = slots // T
        sl = slots % T
        ohT = np.zeros((NBLK * b_tiles, 128, T), np.float32)
        ohT[tidx, w[sel], sl] = 1
        ohE = np.zeros((NBLK * b_tiles, 128, 4, 128), ml_dtypes.bfloat16)
        ohE[tidx, sl % 128, sl // 128, w[sel]] = 1

        m = dict(shared)
        m["featT_grid"] = np.ascontiguousarray(fg.T)
        m["featT_mesh"] = np.ascontiguousarray(fm.T)
        m["featT_src"] = np.ascontiguousarray(fs.T)
        m["eattrT"] = np.ascontiguousarray(ea.T)
        m["ohT"] = ohT
        m["ohE"] = ohE
        in_maps.append(m)

    return in_maps, b_tiles, ln2c


def assemble_output(results, n_grid=N_GRID, n_mesh=N_H3):
    gn = n_grid // NCORES
    x = np.empty((n_grid + n_mesh, HID), np.float32)
    for k in range(NCORES):
        x[k * gn:(k + 1) * gn] = results[k]["out_grid"][:gn]
        lo = k * MESH_PER
        hi = min((k + 1) * MESH_PER, n_mesh)
        if hi > lo:
            x[n_grid + lo:n_grid + hi] = results[k]["out_mesh"][:hi - lo]
    return x


def kernel(features, edge_attr, latent_edge_attr, node_enc, edge_enc,
           ep_mlp, np_mlp, edge_index, latent_edge_index):
    ei = np.asarray(edge_index)
    src, dst = ei[0], ei[1]
    grid_tiles = -(-(N_GRID // NCORES) // T)      # 64
    in_maps, b_tiles, ln2c = host_prep(
        features, edge_attr, node_enc, edge_enc, ep_mlp, np_mlp,
        src, dst, grid_tiles)
    nc = build_program(grid_tiles, b_tiles, ln2c)
    res = run_bass_kernel_spmd(nc, in_maps, list(range(NCORES)))
    x = assemble_output(res.results)
    return (x, np.asarray(latent_edge_index), np.asarray(latent_edge_attr))
